# revision 31
# baseline (speedup 1.0000x reference)
"""Causal self-attention (N=2, S=4096, E=768, H=12) on 8 NeuronCores.

Sharding: batch x head-group. Core c handles batch n = c // 4 and heads
h0 = (c % 4) * 3 .. h0+2 (3 heads per core, 24 (n,h) pairs over 8 cores).

Per-core kernel (SPMD, identical program, per-core input values):
  inputs:  xT   [768, 4096] bf16  x[n] transposed (host layout prep)
           wqk  [3, 768, 128] bf16  per head [Wq_h | Wk_h] column blocks
           wv   [768, 192] bf16     Wv columns for the 3 heads
  output:  outT [3, 64, 4096] f32  per-head attention output, transposed

The device is issue/sync-overhead bound (~0.1us per instruction on the
critical path), so everything is organized around FEW, WIDE instructions:
1024-wide query stripes (the bf16 moving-operand max), one exp ACTIVATE
per 128-key chunk, and a deep PSUM pipeline so cross-engine handoffs
(PE -> ScalarE -> PE) hide behind lookahead.

Per head (flash-style, scores never leave the chip):
  qT/kT [64, 4096] bf16 via matmul + DVE copy (PSUM cannot be DMA'd),
        duplicated into both partition halves so score matmuls can
        row-pack 2 chunks (K=64 uses half the PE array).
  scores TRANSPOSED per key chunk: sT[sk 128, sq 1024] = matmul(
        lhsT=kT-chunk, rhs=qT-stripe) bf16; diagonal chunks stream only
        the causally-live suffix. exp: one ScalarE ACTIVATE (scale=1/8
        folded) with bf16 output (2x the fp32 ACT rate); causal triangle
        zeroed in-place by GPSIMD affine_select; PV consumes exp(sT)
        with lhsT = [v | 1] bf16 so the softmax row-sum rides in PSUM
        row 64.
  division: DVE copy pv->SBUF (frees the PSUM bank), DVE reciprocal of
        row 64 in place, 4KB DMA hop to partition 0, GPSIMD
        partition_broadcast, DVE multiply, DMA out; the five stages are
        drip-fed one per pipeline unit so no in-order queue blocks.
Projection biases are all-zero for this problem; a program with bias
rows folded in (K=1 ones matmuls) is compiled on demand if any bias is
nonzero.
"""

import os
import sys

import numpy as np

for _p in ("/opt/trn_rl_repo",):
    if _p not in sys.path and os.path.isdir(_p):
        sys.path.insert(0, _p)

import concourse.bass as bass  # noqa: E402
import concourse.mybir as mybir  # noqa: E402
import concourse.tile as tile  # noqa: E402
from concourse import bacc  # noqa: E402

F32 = mybir.dt.float32
BF16 = mybir.dt.bfloat16
I16 = mybir.dt.int16

N, S, E, H = 2, 4096, 768, 12
D = 64
HPC = 3  # heads per core
P = 128
SLAB = 512  # PSUM bank = 512 fp32 cols caps every matmul's free dim
CHUNK = 128
GROUP = 3  # chunks per exp batch; one ACTIVATE per group, no partials
LOOKAHEAD = 2  # score groups emitted this far ahead of pv
KCH = E // P  # 6 contraction chunks

# Schraudolph constants for bf16-bit-pattern exp(x/8) on DVE (optional
# rebalancing path; ScalarE bf16-out handles all exp by default).
EXP_A = (128.0 / float(np.log(2.0))) / 8.0
EXP_C2 = -5.09
DVE_NUM, DVE_DEN = 0, 5


def build_nc(seq=S, n_cores=8, reps=1, has_bias=False, abl=()):
    nslab = seq // SLAB
    nchunk = seq // CHUNK
    cps = SLAB // CHUNK  # chunks per stripe

    nc = bacc.Bacc("TRN2", target_bir_lowering=False, debug=False,
                   num_devices=n_cores)

    xT_d = nc.dram_tensor("xT", [E, seq], BF16, kind="ExternalInput")
    wqk_d = nc.dram_tensor("wqk", [HPC, E, P], BF16, kind="ExternalInput")
    wv_d = nc.dram_tensor("wv", [E, HPC * D], BF16, kind="ExternalInput")
    if has_bias:
        bqk_d = nc.dram_tensor("bqk", [HPC, 1, P], BF16, kind="ExternalInput")
        bv_d = nc.dram_tensor("bv", [1, HPC * D], BF16, kind="ExternalInput")
    od = D + 1 if "divhost" in abl else D
    outT_d = nc.dram_tensor("outT", [HPC, od, seq], F32, kind="ExternalOutput")

    xT_r = xT_d.ap().rearrange("(o p) s -> p o s", p=P)
    wqk_r = wqk_d.ap().rearrange("h (o p) m -> p h o m", p=P)
    wv_r = wv_d.ap().rearrange("(o p) m -> p o m", p=P)

    add = mybir.AluOpType.add
    mult = mybir.AluOpType.mult
    Exp = mybir.ActivationFunctionType.Exp

    with tile.TileContext(nc) as tc:
        with (
            tc.tile_pool(name="const", bufs=1) as cpool,
            tc.tile_pool(name="persist", bufs=1) as ppool,
            tc.tile_pool(name="xt", bufs=2) as xtpool,
            tc.tile_pool(name="ework", bufs=5) as epool,
            tc.tile_pool(name="small", bufs=2) as spool,
            # sc tiles [128, GROUP*512] f32 = 3 banks x 2 bufs
            tc.tile_pool(name="psc", bufs=2, space="PSUM") as psc,
            tc.tile_pool(name="ppv", bufs=1, space="PSUM") as ppv,
            tc.tile_pool(name="pproj", bufs=1, space="PSUM") as pproj,
        ):
            # ---- constants; wqk + first x stripe lead the DMA queue ----
            wqk_sb = cpool.tile([P, HPC, KCH, P], BF16)
            nc.sync.dma_start(wqk_sb[:], wqk_r)
            xt_first = xtpool.tile([P, KCH, SLAB], BF16, tag="xt", name="xt")
            nc.sync.dma_start(xt_first[:], xT_r[:, :, 0:SLAB])
            wv_sb = cpool.tile([P, KCH, HPC * D], BF16)
            nc.sync.dma_start(wv_sb[:], wv_r)
            if has_bias:
                bqk_sb = cpool.tile([HPC, 1, P], BF16)
                nc.sync.dma_start(bqk_sb[:], bqk_d.ap())
                bv_sb = cpool.tile([1, HPC * D], BF16)
                nc.sync.dma_start(bv_sb[:], bv_d.ap())
                ones_row = cpool.tile([1, SLAB], BF16)
                nc.vector.memset(ones_row[:], 1.0)

            # dummy exp: ACT table load at t=0, off the critical path
            warm = cpool.tile([1, 1], F32)
            nc.vector.memset(warm[:], 0.0)
            nc.scalar.activation(warm[:], warm[:], Exp)

            # all-ones [65, 64] f32: row 64 is the K=1 lhsT of the rank-1
            # broadcast matmul in the division (base_partition 64).
            F32R = mybir.dt.float32r
            ones65f = cpool.tile([D + 1, D], F32)
            nc.vector.memset(ones65f[:], 1.0)
            ones65 = cpool.tile([D + 1, D], F32R)
            with nc.allow_low_precision(reason="f32r is fp32-width"):
                nc.vector.tensor_copy(ones65[:], ones65f[:])

            # [v | 1] bf16 augmented values: col D carries the row-sum.
            v_aug = cpool.tile([P, nchunk, HPC, D + 1], BF16)
            ones_sb = cpool.tile([P, 1], BF16)
            nc.vector.memset(ones_sb[:], 1.0)
            nc.vector.tensor_copy(
                v_aug[:, :, :, D : D + 1],
                ones_sb[:, None, None, :].to_broadcast((P, nchunk, HPC, 1)),
            )

            # persistent PV staging: all 24 (stripe, head) results live
            # here until the end-of-rep division epilogue
            cpall = cpool.tile([D + 1, nslab * HPC, SLAB], F32)

            qsb = []
            ksb = []
            for h in range(HPC):
                qsb.append(ppool.tile([D, seq], BF16, name=f"q{h}"))
                ksb.append(ppool.tile([D, seq], BF16, name=f"k{h}"))

            def load_xt(j):
                sl = slice(j * SLAB, (j + 1) * SLAB)
                xt = xtpool.tile([P, KCH, SLAB], BF16, tag="xt", name="xt")
                nc.sync.dma_start(xt[:], xT_r[:, :, sl])
                return xt

            def proj_slab(j, xt):
                sl = slice(j * SLAB, (j + 1) * SLAB)
                for h in range(HPC):
                    ps = pproj.tile([P, SLAB], F32, tag="proj")
                    for k in ([] if "noproj" in abl else range(KCH)):
                        nc.tensor.matmul(
                            ps[:],
                            lhsT=wqk_sb[:, h, k, :],
                            rhs=xt[:, k, :],
                            start=(k == 0),
                            stop=(k == KCH - 1) and not has_bias,
                        )
                    if has_bias:
                        nc.tensor.matmul(
                            ps[:], lhsT=bqk_sb[h, :, :], rhs=ones_row[:],
                            start=False, stop=True,
                        )
                    # PSUM -> SBUF (bf16) exit copies, split across the
                    # two engines that can read PSUM
                    nc.scalar.copy(qsb[h][:, sl], ps[0:D, :])
                    nc.vector.tensor_copy(ksb[h][:, sl], ps[D:P, :])
                for c8 in range(cps):
                    c = j * cps + c8
                    pv_ = pproj.tile([P, SLAB], F32, tag="proj")
                    for k in ([] if "noproj" in abl else range(KCH)):
                        nc.tensor.matmul(
                            pv_[:, 0 : HPC * D],
                            lhsT=xt[:, k, c8 * CHUNK : (c8 + 1) * CHUNK],
                            rhs=wv_sb[:, k, :],
                            start=(k == 0),
                            stop=(k == KCH - 1) and not has_bias,
                        )
                    if has_bias:
                        nc.tensor.matmul(
                            pv_[:, 0 : HPC * D],
                            lhsT=ones_sb[0:1, :], rhs=bv_sb[:],
                            start=False, stop=True,
                        )
                    nc.vector.tensor_copy(
                        v_aug[:, c, :, 0:D],
                        pv_[:, 0 : HPC * D].rearrange("p (h d) -> p h d", h=HPC),
                    )

            gctr = [0]  # exp engine assignment counter

            def attn_units(h, j):
                """Per-chunk (scores, pv, tail) units for one head's
                stripe; pipelined across heads by the caller."""
                nch = (j + 1) * cps  # causal: key chunks 0 .. (j+1)*cps-1
                state = {}

                def scores_group(g):
                    c0 = g * GROUP
                    cn = min(GROUP, nch - c0)
                    sc = psc.tile([P, GROUP * SLAB], F32, tag="sc", name="sc")
                    for ci in ([] if "noscores" in abl else
                               range(c0, c0 + cn)):
                        off = (ci - c0) * SLAB
                        m = ci - j * cps
                        lo = CHUNK * m if m >= 1 else 0
                        nc.tensor.matmul(
                            sc[:, off + lo : off + SLAB],
                            lhsT=ksb[h][:, ci * CHUNK : (ci + 1) * CHUNK],
                            rhs=qsb[h][:, j * SLAB + lo : (j + 1) * SLAB],
                            start=True,
                            stop=True,
                        )
                    et = epool.tile([P, GROUP * SLAB], BF16, tag="E",
                                    name="et")
                    state[g] = (et, c0, cn)
                    if "noexp" in abl:
                        return
                    use_dve = (gctr[0] % DVE_DEN) < DVE_NUM
                    gctr[0] += 1
                    # ONE op over the whole group, including the causally
                    # dead prefix of diagonal chunks: exp(garbage) lands in
                    # et ranges the PV matmuls never read, and one wide op
                    # is cheaper than several narrow ones.
                    if use_dve:
                        nc.vector.tensor_scalar(
                            et[:, : cn * SLAB].bitcast(I16),
                            sc[:, : cn * SLAB],
                            EXP_A, 127.0 * 128 + EXP_C2, mult, add,
                        )
                    else:
                        nc.scalar.activation(
                            et[:, : cn * SLAB], sc[:, : cn * SLAB],
                            Exp, scale=0.125,
                        )
                    for ci in range(c0, c0 + cn):
                        m = ci - j * cps
                        off = (ci - c0) * SLAB
                        if m >= 0:  # diagonal: zero the sq < sk triangle
                            nc.gpsimd.affine_select(
                                out=et[:, off + CHUNK * m : off + CHUNK * (m + 1)],
                                in_=et[:, off + CHUNK * m : off + CHUNK * (m + 1)],
                                compare_op=mybir.AluOpType.is_ge,
                                fill=0.0,
                                base=0,
                                pattern=[[1, CHUNK]],
                                channel_multiplier=-1,
                            )

                def pv_group(g):
                    if "nopv" in abl:
                        return
                    if g == 0:
                        state["pv"] = ppv.tile([D + 1, SLAB], F32, tag="pv",
                                               name="pv")
                    et, c0, cn = state[g]
                    for ci in range(c0, c0 + cn):
                        off = (ci - c0) * SLAB
                        m = ci - j * cps
                        lo = CHUNK * m if m >= 1 else 0
                        nc.tensor.matmul(
                            state["pv"][:, lo:SLAB],
                            lhsT=v_aug[:, ci, h, :],
                            rhs=et[:, off + lo : off + SLAB],
                            start=(ci == 0),
                            stop=(ci == nch - 1),
                            skip_group_check=True,
                        )

                def cp_fn():
                    # copy PV out of PSUM immediately: frees the single
                    # ppv bank for the next head
                    nc.vector.tensor_copy(cpall[:, j * HPC + h, :],
                                          state["pv"][:])

                ngrp = (nch + GROUP - 1) // GROUP
                units = [
                    [(lambda g=g: scores_group(g)),
                     (lambda g=g: pv_group(g)), None]
                    for g in range(ngrp)
                ]
                if "nopv" in abl:
                    return units, None
                if "nodiv" in abl:
                    units[-1][2] = [cp_fn]  # still frees the ppv bank
                    return units, None
                if "divhost" in abl:
                    def out_fn(sl=slice(j * SLAB, (j + 1) * SLAB)):
                        nc.sync.dma_start(outT_d.ap()[h, :, sl],
                                          cpall[:, j * HPC + h, :])
                    units[-1][2] = [cp_fn, out_fn]
                    return units, None
                units[-1][2] = [cp_fn]
                return units, True

            # Emission: scores run LOOKAHEAD chunks ahead of pv; division
            # stages and the next stripe's projection are drip-fed
            # between units.
            for _rep in range(reps):
                xt_cur = xt_first if _rep == 0 else load_xt(0)
                proj_slab(0, xt_cur)
                for j in range(nslab):
                    units = []
                    divs = []
                    for h in range(HPC):
                        u, dv = attn_units(h, j)
                        units.extend(u)
                        if dv is not None:
                            divs.append(dv)
                    nun = len(units)
                    pending = []
                    proj_at = nun // 2
                    if j + 1 < nslab:
                        xt_cur = load_xt(j + 1)  # in flight during attn(j)
                    for i in range(min(LOOKAHEAD, nun)):
                        units[i][0]()
                    for i, (_, pv_fn, tail) in enumerate(units):
                        if i + LOOKAHEAD < nun:
                            units[i + LOOKAHEAD][0]()
                        pv_fn()
                        if tail is not None:
                            pending.extend(tail)
                        if pending:
                            pending.pop(0)()
                        if i == proj_at and j + 1 < nslab:
                            proj_slab(j + 1, xt_cur)
                    for fn in pending:
                        fn()
                if "nopv" in abl or "nodiv" in abl or "divhost" in abl:
                    continue
                # end-of-rep division epilogue: all reciprocal rows in one
                # burst, then wave-pipelined broadcast/multiply/out-DMA.
                # The chain latency is paid once per rep, and in sustained
                # execution the whole epilogue overlaps the next rep's
                # PE/ACT-bound attention.
                nidx = nslab * HPC
                for i in range(nidx):
                    nc.vector.reciprocal(cpall[D : D + 1, i, :],
                                         cpall[D : D + 1, i, :])
                WAVE = 4
                for w0 in range(0, nidx, WAVE):
                    wave = range(w0, min(w0 + WAVE, nidx))
                    recs = {}
                    for i in wave:
                        rec0 = spool.tile([1, SLAB], F32, tag="rec0",
                                          name="rec0", bufs=4)
                        nc.sync.dma_start(rec0[:], cpall[D : D + 1, i, :])
                        recs[i] = rec0
                    rbcs = {}
                    for i in wave:
                        rbc = spool.tile([D, SLAB], F32, tag="rbc",
                                         name="rbc", bufs=4)
                        nc.gpsimd.partition_broadcast(rbc[:], recs[i][:])
                        rbcs[i] = rbc
                    for i in wave:
                        jj, hh = divmod(i, HPC)
                        osb = spool.tile([D, SLAB], F32, tag="osb",
                                         name="osb", bufs=4)
                        nc.vector.tensor_tensor(
                            osb[:], cpall[0:D, i, :], rbcs[i][:], mult
                        )
                        nc.sync.dma_start(
                            outT_d.ap()[hh, :,
                                        jj * SLAB : (jj + 1) * SLAB],
                            osb[:],
                        )

    nc.compile()
    return nc


def shard_inputs(x, Wq, bq, Wk, bk, Wv, bv, n_cores=8, hpc=HPC):
    """Host-side layout prep: slice per-core head groups + transpose x."""
    import ml_dtypes

    bf16 = ml_dtypes.bfloat16
    in_maps = []
    nb = x.shape[0]
    groups = n_cores // nb  # head groups per batch
    has_bias = bool(np.any(bq) or np.any(bk) or np.any(bv))
    xT = [np.ascontiguousarray(x[n].T).astype(bf16) for n in range(nb)]
    for core in range(n_cores):
        n = core // groups
        h0 = (core % groups) * hpc
        wqk = np.stack(
            [
                np.concatenate(
                    [
                        Wq[:, (h0 + i) * D : (h0 + i + 1) * D],
                        Wk[:, (h0 + i) * D : (h0 + i + 1) * D],
                    ],
                    axis=1,
                )
                for i in range(hpc)
            ]
        ).astype(bf16)
        m = {
            "xT": xT[n],
            "wqk": np.ascontiguousarray(wqk),
            "wv": np.ascontiguousarray(
                Wv[:, h0 * D : (h0 + hpc) * D].astype(bf16)
            ),
        }
        if has_bias:
            bqk = np.stack(
                [
                    np.concatenate(
                        [bq[(h0 + i) * D : (h0 + i + 1) * D],
                         bk[(h0 + i) * D : (h0 + i + 1) * D]]
                    )[None, :]
                    for i in range(hpc)
                ]
            ).astype(bf16)
            m["bqk"] = np.ascontiguousarray(bqk)
            m["bv"] = np.ascontiguousarray(
                bv[None, h0 * D : (h0 + hpc) * D].astype(bf16)
            )
        in_maps.append(m)
    return in_maps


def gather_output(results, n_cores=8, nb=N, seq=S, emb=E, hpc=HPC):
    out = np.empty((nb, seq, emb), np.float32)
    groups = n_cores // nb
    for core in range(n_cores):
        n = core // groups
        h0 = (core % groups) * hpc
        oT = results[core]["outT"]  # [hpc, D, seq]
        for i in range(hpc):
            out[n, :, (h0 + i) * D : (h0 + i + 1) * D] = oT[i].T
    return out


_NC_CACHE = {}


def _get_nc(has_bias=False):
    key = ("nc", has_bias)
    if key not in _NC_CACHE:
        _NC_CACHE[key] = build_nc(has_bias=has_bias)
    return _NC_CACHE[key]


def run_on_hw(inputs, trace=False):
    """Run on the 8 NeuronCores; returns (full_output, BassKernelResults)."""
    from concourse.bass_utils import run_bass_kernel_spmd

    in_maps = shard_inputs(**inputs)
    nc = _get_nc(has_bias="bqk" in in_maps[0])
    res = run_bass_kernel_spmd(nc, in_maps, list(range(8)), trace=trace)
    return gather_output(res.results), res


def kernel(x, Wq, bq, Wk, bk, Wv, bv):
    x = np.asarray(x)
    out, _ = run_on_hw(
        dict(x=x, Wq=np.asarray(Wq), bq=np.asarray(bq), Wk=np.asarray(Wk),
             bk=np.asarray(bk), Wv=np.asarray(Wv), bv=np.asarray(bv))
    )
    return out.astype(np.float32)
